# revision 10
# baseline (speedup 1.0000x reference)
"""Trainium2 Bass kernel for BertWithAdaThresholdLocContextPooling.

Strategy: pure data parallel over batch (B=16 -> 2 batches per core x 8 cores).

Restructured from the 68us baseline for overlap and PE efficiency:
  - gather indices computed on the HOST and packed into ONE tiny DMA
    (dma_start costs ~0.7us of issue time on its engine, so DMA count
    on the critical engines is minimized),
  - bulk weight loads issue from the sync engine as two parallel chains
    (a single DMA stream only sustains ~100-150GB/s; two keep the HBM
    path saturated); small consts issue from the scalar engine so the
    activation-table loads are not queued behind bulk issue,
  - attention pooling runs in column space: transpose the 96 gathered
    attention rows once on the PE, then mention-mean / head-product /
    head-mean are full-width DVE ops,
  - rs = seq^T @ ht uses ht as the stationary operand (8 matmuls of
    N<=512 per batch instead of 48 matmuls of N=1),
  - ht normalization is folded out of the critical path,
  - extractor accumulation is split into arrival-ordered phases that
    pipeline with the weight DMA chains; classifier is split so its
    first half starts as soon as wbsA lands.
"""

import sys

for _p in ("/opt/trn_rl_repo",):
    if _p not in sys.path:
        sys.path.insert(0, _p)

import numpy as np
import ml_dtypes

import concourse.bacc as bacc
import concourse.bass as bass
import concourse.mybir as mybir
from concourse.tile import TileContext
from concourse.bass_utils import run_bass_kernel_spmd
from concourse.tile_rust import add_dep_helper

F32 = mybir.dt.float32
BF16 = mybir.dt.bfloat16
I32 = mybir.dt.int32
AF = mybir.ActivationFunctionType
ALU = mybir.AluOpType
AX = mybir.AxisListType

B, L, HID = 16, 512, 768
HEADS, M = 12, 4
EMB, BLK, NER, NCLS = 768, 8, 6, 97
NCORES = 8
BPC = B // NCORES          # batches per core = 2
CAT = 2 * HID + NER        # 1542
NEMB = EMB // 128          # 6 chunks of EMB
NL = L // 128              # 4 chunks of L
NBL = EMB * BLK // 128     # 48 classifier contraction chunks
HALVES = ((0, 384), (384, 384))

# cbs bf16 [128, 103]: id96 | selE | ones | sel2
CBS_ID96 = 0
CBS_SELE = 96
CBS_ONES = 100
CBS_SEL2 = 101
CBS_COLS = 103
# cbias bf16 [1, 1544]: bh | selbh | bt | selbt (single partition row)
CB_BH = 0
CB_SELBH = EMB
CB_BT = EMB + 4
CB_SELBT = 2 * EMB + 4
CB_COLS = 2 * EMB + 8
# smallf f32 [97, 10]: id4 | bb col | ner [6,4]
SMF_BB = 4
SMF_NER = 5
SMF_COLS = 10

_cache = {}


def _build_constants():
    cbs = np.zeros((128, CBS_COLS), ml_dtypes.bfloat16)
    cbs[0:96, 0:96] = np.eye(96)
    for k in range(4 * M):
        cbs[k, CBS_SELE + k // M] = 1.0
    cbs[0:128, CBS_ONES] = 1.0
    cbs[0:4, CBS_SEL2] = 1.0
    cbs[4:8, CBS_SEL2 + 1] = 1.0

    rys = np.zeros((128, BLK * 128), ml_dtypes.bfloat16)
    for y in range(BLK):
        for p in range(128):
            rys[(p // BLK) * BLK + y, y * 128 + p] = 1.0

    perm = np.empty(EMB * BLK, np.int64)
    for cch in range(NEMB):
        for y in range(BLK):
            for p in range(128):
                g = cch * 16 + p // BLK
                x = p % BLK
                perm[(cch * BLK + y) * 128 + p] = g * 64 + x * BLK + y
    return {"cbs": cbs, "rys": rys, "perm": perm}


def _build_program():
    nc = bacc.Bacc("TRN2", target_bir_lowering=False, debug=False)

    seq_h = nc.dram_tensor("seq", [BPC * L, HID], BF16, kind="ExternalInput")
    attn_h = nc.dram_tensor("attn", [BPC * HEADS * L, L], BF16, kind="ExternalInput")
    idxp_h = nc.dram_tensor("idxp", [128, 2], I32, kind="ExternalInput")
    smf_h = nc.dram_tensor("smf", [NCLS, SMF_COLS], F32, kind="ExternalInput")
    cbs_h = nc.dram_tensor("cbs", [128, CBS_COLS], BF16, kind="ExternalInput")
    cbias_h = nc.dram_tensor("cbias", [1, CB_COLS], BF16, kind="ExternalInput")
    rys_h = nc.dram_tensor("rys", [128, BLK * 128], BF16, kind="ExternalInput")
    whsA_h = nc.dram_tensor("whsA", [128, 6 * EMB], BF16, kind="ExternalInput")
    whsB_h = nc.dram_tensor("whsB", [128, 6 * EMB], BF16, kind="ExternalInput")
    whn_h = nc.dram_tensor("whn", [NER, EMB], BF16, kind="ExternalInput")
    wtsA_h = nc.dram_tensor("wtsA", [128, 6 * EMB], BF16, kind="ExternalInput")
    wtsB_h = nc.dram_tensor("wtsB", [128, 6 * EMB], BF16, kind="ExternalInput")
    wtn_h = nc.dram_tensor("wtn", [NER, EMB], BF16, kind="ExternalInput")
    wbsA_h = nc.dram_tensor("wbsA", [128, 24 * NCLS], BF16, kind="ExternalInput")
    wbsB_h = nc.dram_tensor("wbsB", [128, 24 * NCLS], BF16, kind="ExternalInput")
    out_h = nc.dram_tensor("logitsT", [NCLS, BPC], F32, kind="ExternalOutput")

    with TileContext(nc) as tc:
        with (
            tc.tile_pool(name="const", bufs=1) as cp,
            tc.tile_pool(name="data", bufs=1) as dp,
            tc.tile_pool(name="psbig", bufs=1, space="PSUM") as psb,
            tc.tile_pool(name="psrs", bufs=1, space="PSUM") as psr,
            tc.tile_pool(name="pssm", bufs=1, space="PSUM") as pss,
        ):
            # ---- sync engine: seq first, then the two bulk chains ----
            seqt = []
            seq_dmas = []
            for b in range(BPC):
                t = dp.tile([128, NL * HID], BF16, tag=f"seq{b}")
                seq_dmas.append(nc.sync.dma_start(
                    t[:].rearrange("p (c d) -> p c d", c=NL),
                    seq_h[b * L:(b + 1) * L, :].rearrange("(c p) d -> p c d", p=128)))
                seqt.append(t)
            whsA = cp.tile([128, 6 * EMB], BF16)
            d_whsA = nc.sync.dma_start(whsA[:], whsA_h[:])
            wtsA = cp.tile([128, 6 * EMB], BF16)
            d_wtsA = nc.sync.dma_start(wtsA[:], wtsA_h[:])
            whsB = cp.tile([128, 6 * EMB], BF16)
            d_whsB = nc.sync.dma_start(whsB[:], whsB_h[:])
            wtsB = cp.tile([128, 6 * EMB], BF16)
            d_wtsB = nc.sync.dma_start(wtsB[:], wtsB_h[:])
            rys = cp.tile([128, BLK * 128], BF16)
            d_rys = nc.sync.dma_start(rys[:], rys_h[:])
            wbsA = cp.tile([128, 24 * NCLS], BF16)
            d_wbsA = nc.sync.dma_start(wbsA[:], wbsA_h[:])
            wbsB = cp.tile([128, 24 * NCLS], BF16)
            d_wbsB = nc.sync.dma_start(wbsB[:], wbsB_h[:])
            # two chains keep >=2 bulk streams in flight the whole window
            for a, b2 in ((d_whsB, d_whsA), (d_wbsA, d_whsB),
                          (d_wtsB, d_wtsA), (d_rys, d_wtsB), (d_wbsB, d_rys)):
                add_dep_helper(a.ins, b2.ins, reason="stagger bulk DMA bandwidth")

            # ---- scalar engine: packed small loads, then activations ----
            idxp = dp.tile([128, 2], I32)
            d_idxp = nc.scalar.dma_start(idxp[:], idxp_h[:])
            cbs = cp.tile([128, CBS_COLS], BF16)
            nc.scalar.dma_start(cbs[:], cbs_h[:])
            cbias = cp.tile([1, CB_COLS], BF16)
            nc.scalar.dma_start(cbias[:], cbias_h[:])
            smf = cp.tile([NCLS, SMF_COLS], F32)
            nc.scalar.dma_start(smf[:], smf_h[:])
            whn = cp.tile([NER, EMB], BF16)
            nc.scalar.dma_start(whn[:], whn_h[:])
            wtn = cp.tile([NER, EMB], BF16)
            nc.scalar.dma_start(wtn[:], wtn_h[:])

            id96 = cbs[0:96, CBS_ID96:CBS_ID96 + 96]
            selE = cbs[0:16, CBS_SELE:CBS_SELE + 4]
            ones = cbs[0:128, CBS_ONES:CBS_ONES + 1]
            sel2 = cbs[0:8, CBS_SEL2:CBS_SEL2 + 2]
            id4f = smf[0:4, 0:4]
            bbc = smf[0:NCLS, SMF_BB:SMF_BB + 1]
            ner4f = smf[0:NER, SMF_NER:SMF_NER + 4]
            bhr = cbias[0:1, CB_BH:CB_BH + EMB]
            btr = cbias[0:1, CB_BT:CB_BT + EMB]
            selbh = cbias[0:1, CB_SELBH:CB_SELBH + 4]
            selbt = cbias[0:1, CB_SELBT:CB_SELBT + 4]

            # ---- gathers (gpsimd; issue as soon as idxp lands) ----
            sg = dp.tile([4 * M, HID], BF16)
            gather_ins = [nc.gpsimd.indirect_dma_start(
                out=sg[:], out_offset=None, in_=seq_h[:],
                in_offset=bass.IndirectOffsetOnAxis(ap=idxp[0:16, 0:1], axis=0))]
            at = []
            for b in range(BPC):
                t = dp.tile([2 * M * HEADS, L], BF16, tag=f"at{b}")
                off = idxp[16:112, 0:1] if b == 0 else idxp[0:96, 1:2]
                gather_ins.append(nc.gpsimd.indirect_dma_start(
                    out=t[:], out_offset=None, in_=attn_h[:],
                    in_offset=bass.IndirectOffsetOnAxis(ap=off, axis=0)))
                at.append(t)

            # bulk weight chains yield the DMA path to the first gathers
            for g in gather_ins[:2]:
                add_dep_helper(d_whsA.ins, g.ins,
                               reason="bulk yields DMA bandwidth to gathers")
                add_dep_helper(d_wtsA.ins, g.ins,
                               reason="bulk yields DMA bandwidth to gathers")
            # seq yields the first ~1us to the tiny idxp load
            for dseq in seq_dmas:
                add_dep_helper(dseq.ins, d_idxp.ins,
                               reason="seq yields DMA bandwidth to idxp")

            ner4 = dp.tile([NER, 4], BF16)
            nc.vector.tensor_copy(ner4[:], ner4f)

            # PE p-state warm-up bridge (full clock needs ~3us continuous)
            ps_warm = pss.tile([96, 96], F32, tag="s1", name="ps_warm")
            for w in range(12):
                nc.tensor.matmul(ps_warm[:], lhsT=id96, rhs=id96,
                                 start=(w == 0), stop=(w == 11))

            # ---- attention pooling in column space ----
            # atT free layout: (b, c, q) with q = e*48 + m*12 + h
            atT = pss.tile([128, BPC * NL * 96], BF16, tag="s0")
            for b in range(BPC):
                for c in range(NL):
                    nc.tensor.transpose(
                        atT[:, (b * NL + c) * 96:(b * NL + c) * 96 + 96],
                        at[b][:, c * 128:(c + 1) * 128], id96)

            # ---- entity embeddings: log-sum-exp over mentions ----
            exps = dp.tile([4 * M, HID], BF16)
            nc.scalar.activation(exps[:], sg[:], AF.Exp)
            ps_e = [psb.tile([4, 384], F32, tag=f"wh{i}", name=f"ps_e{i}")
                    for i in range(2)]
            ent = dp.tile([4, HID], BF16)
            for i, (n0, nl_) in enumerate(HALVES):
                nc.tensor.matmul(ps_e[i][:], lhsT=selE,
                                 rhs=exps[:, n0:n0 + nl_], start=True, stop=True)
                nc.scalar.activation(ent[:, n0:n0 + nl_], ps_e[i][:], AF.Ln)

            # mean over mentions (scales folded into the final normalization)
            ea = dp.tile([128, 8 * 2 * HEADS], F32)
            nc.vector.tensor_reduce(
                ea[:].rearrange("p (bc e h) -> p bc e h", bc=8, e=2),
                atT[:].rearrange("p (bc e m h) -> p bc e h m", bc=8, e=2, m=M),
                axis=AX.X, op=ALU.add)
            hh = dp.tile([128, 8 * HEADS], BF16)
            nc.vector.tensor_tensor(
                out=hh[:].rearrange("p (bc h) -> p bc h", bc=8),
                in0=ea[:].rearrange("p (bc e h) -> p bc e h", bc=8, e=2)[:, :, 0],
                in1=ea[:].rearrange("p (bc e h) -> p bc e h", bc=8, e=2)[:, :, 1],
                op=ALU.mult)
            htr = dp.tile([128, 8], F32)
            nc.vector.tensor_reduce(
                htr[:].rearrange("p (bc one) -> p bc one", one=1),
                hh[:].rearrange("p (bc h) -> p bc h", bc=8),
                axis=AX.X, op=ALU.add)
            htc = dp.tile([128, 8], BF16)
            nc.vector.tensor_copy(htc[:], htr[:])

            # ---- denominator path (off the rs critical path) ----
            ps_d8 = pss.tile([8, 1], F32, tag="s1")
            nc.tensor.matmul(ps_d8[:], lhsT=htc[:], rhs=ones, start=True, stop=True)
            d8 = dp.tile([8, 1], BF16)
            nc.vector.tensor_copy(d8[:], ps_d8[:])
            ps_d2 = pss.tile([1, 2], F32, tag="s1")
            nc.tensor.matmul(ps_d2[:], lhsT=d8[:], rhs=sel2, start=True, stop=True)
            den2 = dp.tile([1, 2], F32)
            # den = sum(ht_raw) + M^2*HEADS*1e-5  (all scales folded here)
            nc.vector.tensor_scalar_add(den2[:], ps_d2[:], M * M * HEADS * 1e-5)
            rcp = dp.tile([1, 2], F32)
            nc.vector.reciprocal(rcp[:], den2[:])

            # ---- rs (unnormalized) = seq^T @ ht_raw, then rescale ----
            rsraw = dp.tile([1, BPC * HID], F32)
            ps_rst = pss.tile([128, 2 * NEMB], F32, tag="s0")
            for b in range(BPC):
                ps_rs = psr.tile([1, HID], F32, tag="rs", name=f"ps_rs{b}")
                for n0, nl_ in ((0, 512), (512, 256)):
                    for c in range(NL):
                        nc.tensor.matmul(
                            ps_rs[:, n0:n0 + nl_],
                            lhsT=htc[:, b * NL + c:b * NL + c + 1],
                            rhs=seqt[b][:, c * HID + n0:c * HID + n0 + nl_],
                            start=(c == 0), stop=(c == NL - 1))
                if b == 0:
                    nc.scalar.activation(
                        rsraw[0:1, 0:HID], ps_rs[:], AF.Copy,
                        scale=rcp[0:1, 0:1])
                else:
                    nc.vector.tensor_scalar_mul(
                        rsraw[0:1, b * HID:(b + 1) * HID], ps_rs[:],
                        rcp[0:1, b:b + 1])
                for c in range(NEMB):
                    nc.tensor.transpose(
                        ps_rst[:, c * 2 + b:c * 2 + b + 1],
                        rsraw[0:1, b * HID + c * 128:b * HID + (c + 1) * 128],
                        id4f[0:1, 0:1])

            # ---- ent transposes to columns (after Ln, before extractor) ----
            ps_et = pss.tile([128, 4 * NEMB], BF16, tag="s1")
            for c in range(NEMB):
                nc.tensor.transpose(ps_et[:, c * 4:(c + 1) * 4],
                                    ent[:, c * 128:(c + 1) * 128], id96[0:4, 0:4])
            entT = dp.tile([128, 4 * NEMB], BF16)
            nc.vector.tensor_copy(entT[:], ps_et[:])

            rsc = dp.tile([128, 4 * NEMB], BF16)
            nc.vector.tensor_copy(
                rsc[:].rearrange("p (r b m) -> p r b m", r=NEMB, b=BPC),
                ps_rst[:].rearrange("p (r b) -> p r b", r=NEMB)
                .unsqueeze(3).broadcast_to([128, NEMB, BPC, 2]))

            # ---- extractor GEMMs, pipelined with the weight DMA chains ----
            ps_wh = [psb.tile([4, 384], F32, tag=f"wh{i}", name=f"ps_wh{i}")
                     for i in range(2)]
            ps_wt = [psb.tile([4, 384], F32, tag=f"wt{i}", name=f"ps_wt{i}")
                     for i in range(2)]
            exts = ((ps_wh, whsA, whsB, whn, selbh, bhr),
                    (ps_wt, wtsA, wtsB, wtn, selbt, btr))
            # phase A: ner + bias + hs chunks (needs only *A weights + entT)
            for ps_w, wA, wB, wn, selb, br in exts:
                for i, (n0, nl_) in enumerate(HALVES):
                    nc.tensor.matmul(ps_w[i][:], lhsT=ner4[:],
                                     rhs=wn[:, n0:n0 + nl_], start=True, stop=False)
                    nc.tensor.matmul(ps_w[i][:], lhsT=selb,
                                     rhs=br[:, n0:n0 + nl_], start=False, stop=False)
                    for j in range(NEMB):
                        nc.tensor.matmul(
                            ps_w[i][:], lhsT=entT[:, j * 4:(j + 1) * 4],
                            rhs=wA[:, j * EMB + n0:j * EMB + n0 + nl_],
                            start=False, stop=False)
            # phase B: rs chunks (needs *B weights + rsc)
            t4 = []
            for wi, (ps_w, wA, wB, wn, selb, br) in enumerate(exts):
                t = dp.tile([4, EMB], BF16, tag=f"t4_{wi}")
                for i, (n0, nl_) in enumerate(HALVES):
                    for j in range(NEMB):
                        nc.tensor.matmul(
                            ps_w[i][:], lhsT=rsc[:, j * 4:(j + 1) * 4],
                            rhs=wB[:, j * EMB + n0:j * EMB + n0 + nl_],
                            start=False, stop=(j == NEMB - 1))
                    nc.scalar.activation(t[:, n0:n0 + nl_], ps_w[i][:], AF.Tanh)
                t4.append(t)

            # ---- transpose hs2/ts2 to columns ----
            ps_a = pss.tile([128, 4 * NEMB], BF16, tag="s0")
            ps_b2 = pss.tile([128, 4 * NEMB], BF16, tag="s1")
            for c in range(NEMB):
                nc.tensor.transpose(ps_a[:, c * 4:(c + 1) * 4],
                                    t4[0][:, c * 128:(c + 1) * 128], id96[0:4, 0:4])
                nc.tensor.transpose(ps_b2[:, c * 4:(c + 1) * 4],
                                    t4[1][:, c * 128:(c + 1) * 128], id96[0:4, 0:4])
            h2t = dp.tile([128, 4 * NEMB], BF16)
            nc.vector.tensor_copy(
                h2t[:].rearrange("p (c b) -> p c b", c=NEMB)[:, :, 0:4:2],
                ps_a[:].rearrange("p (c b) -> p c b", c=NEMB)[:, :, 0:4:2])
            nc.vector.tensor_copy(
                h2t[:].rearrange("p (c b) -> p c b", c=NEMB)[:, :, 1:4:2],
                ps_b2[:].rearrange("p (c b) -> p c b", c=NEMB)[:, :, 1:4:2])

            # ---- grouped bilinear ----
            # ts-replication: out col layout (y, c, b) = y*12 + c*2 + b
            ps_t2x = pss.tile([128, BLK * NEMB * BPC], F32, tag="s1")
            tscols = h2t[:].rearrange("p (c b) -> p c b", c=NEMB)[:, :, 1:4:2]
            for y in range(BLK):
                nc.tensor.matmul(
                    ps_t2x[:, y * 12:(y + 1) * 12]
                    .rearrange("p (c b) -> p c b", c=NEMB),
                    lhsT=rys[:, y * 128:(y + 1) * 128],
                    rhs=tscols, start=True, stop=True)
            # blt col layout (c, y, b)
            blt = dp.tile([128, NEMB * 16], BF16)
            nc.vector.tensor_tensor(
                out=blt[:].rearrange("p (c y b) -> p c y b", c=NEMB, y=BLK),
                in0=h2t[:].rearrange("p (c b) -> p c b", c=NEMB)[:, :, 0:4:2]
                .unsqueeze(3).broadcast_to([128, NEMB, BPC, BLK])
                .rearrange("p c b y -> p c y b"),
                in1=ps_t2x[:].rearrange("p (y c b) -> p c y b", y=BLK, c=NEMB),
                op=ALU.mult)

            # ---- classifier (split so half A starts when wbsA lands) ----
            ps_l = pss.tile([NCLS, BPC], F32, tag="s0")
            for half, wbs in ((0, wbsA), (1, wbsB)):
                for kk in range(24):
                    k = half * 24 + kk
                    nc.tensor.matmul(ps_l[:], lhsT=wbs[:, kk * NCLS:(kk + 1) * NCLS],
                                     rhs=blt[:, k * 2:k * 2 + 2],
                                     start=(k == 0), stop=(k == NBL - 1))
            lg = dp.tile([NCLS, BPC], F32)
            nc.vector.tensor_scalar_add(lg[:], ps_l[:], bbc[:, :1])
            nc.scalar.dma_start(out_h[:], lg[:])

    nc.finalize()
    return nc


def _get_program():
    if "nc" not in _cache:
        _cache["nc"] = _build_program()
        _cache["consts"] = _build_constants()
    return _cache["nc"], _cache["consts"]


def kernel(sequence_output, attention, entity_pos, hs_ner_tags, ts_ner_tags,
           Wh, bh, Wt, bt, Wb, bb):
    nc, c = _get_program()

    seq = np.asarray(sequence_output, dtype=np.float32).astype(ml_dtypes.bfloat16)
    attn = np.asarray(attention, dtype=np.float32).astype(ml_dtypes.bfloat16)
    pos = np.asarray(entity_pos).astype(np.int64)
    nh = np.asarray(hs_ner_tags, dtype=np.float32)
    nt = np.asarray(ts_ner_tags, dtype=np.float32)
    whT = np.ascontiguousarray(np.asarray(Wh, dtype=np.float32).T).astype(ml_dtypes.bfloat16)
    wtT = np.ascontiguousarray(np.asarray(Wt, dtype=np.float32).T).astype(ml_dtypes.bfloat16)
    wbT = np.ascontiguousarray(np.asarray(Wb, dtype=np.float32).T)[c["perm"]]
    wbT = wbT.astype(ml_dtypes.bfloat16)

    def wimg(w, lo):
        # [128, (j, EMB)] image of contraction chunks lo..lo+5
        blk = w[lo * 128:(lo + 6) * 128]
        return np.ascontiguousarray(
            blk.reshape(6, 128, EMB).transpose(1, 0, 2).reshape(128, 6 * EMB))

    whsA, whsB = wimg(whT, 0), wimg(whT, 6)
    wtsA, wtsB = wimg(wtT, 0), wimg(wtT, 6)
    whn = np.ascontiguousarray(whT[12 * 128:CAT])
    wtn = np.ascontiguousarray(wtT[12 * 128:CAT])
    wbp = wbT.reshape(NBL, 128, NCLS).transpose(1, 0, 2)
    wbsA = np.ascontiguousarray(wbp[:, 0:24].reshape(128, 24 * NCLS))
    wbsB = np.ascontiguousarray(wbp[:, 24:48].reshape(128, 24 * NCLS))

    cbias = np.zeros((1, CB_COLS), ml_dtypes.bfloat16)
    cbias[0, CB_BH:CB_BH + EMB] = np.asarray(bh, np.float32)
    cbias[0, CB_BT:CB_BT + EMB] = np.asarray(bt, np.float32)
    cbias[0, CB_SELBH:CB_SELBH + 4] = np.array([1.0, 0.0, 1.0, 0.0])
    cbias[0, CB_SELBT:CB_SELBT + 4] = np.array([0.0, 1.0, 0.0, 1.0])

    in_maps = []
    for core in range(NCORES):
        b0 = core * BPC
        pc = pos[b0:b0 + BPC]                                # [2,2,M]
        ner = np.stack([nh[b0], nt[b0], nh[b0 + 1], nt[b0 + 1]], axis=1)
        smf = np.zeros((NCLS, SMF_COLS), np.float32)
        smf[0:4, 0:4] = np.eye(4)
        smf[0:NCLS, SMF_BB] = np.asarray(bb, np.float32)
        smf[0:NER, SMF_NER:SMF_NER + 4] = ner
        # idxp col0: seq gather rows (16) then attn rows for b=0 (96);
        # col1: attn rows for b=1
        idxp = np.zeros((128, 2), np.int32)
        for k in range(4 * M):
            b, e, m = k // 8, (k // 4) % 2, k % 4
            idxp[k, 0] = b * L + int(pc[b, e, m]) + 1
        for b in range(BPC):
            for q in range(2 * M * HEADS):
                e, m, h = q // 48, (q // 12) % 4, q % 12
                v = (b * HEADS + h) * L + int(pc[b, e, m]) + 1
                if b == 0:
                    idxp[16 + q, 0] = v
                else:
                    idxp[q, 1] = v
        im = {
            "seq": np.ascontiguousarray(seq[b0:b0 + BPC]).reshape(BPC * L, HID),
            "attn": np.ascontiguousarray(attn[b0:b0 + BPC]).reshape(BPC * HEADS * L, L),
            "idxp": idxp,
            "smf": smf,
            "whsA": whsA, "whsB": whsB, "whn": whn,
            "wtsA": wtsA, "wtsB": wtsB, "wtn": wtn,
            "wbsA": wbsA, "wbsB": wbsB,
            "cbs": c["cbs"], "cbias": cbias, "rys": c["rys"],
        }
        in_maps.append(im)

    res = run_bass_kernel_spmd(nc, in_maps, core_ids=list(range(NCORES)))
    _cache["last_res"] = res
    out = np.empty((B, NCLS), np.float32)
    for core in range(NCORES):
        out[core * BPC:(core + 1) * BPC] = res.results[core]["logitsT"].T
    return out


# revision 11
# speedup vs baseline: 1.1313x; 1.1313x over previous
"""Trainium2 Bass kernel for BertWithAdaThresholdLocContextPooling.

Strategy: pure data parallel over batch (B=16 -> 2 batches per core x 8 cores).

Restructured from the 68us baseline for overlap and PE efficiency:
  - gather indices computed on the HOST and packed into ONE tiny DMA
    (dma_start costs ~0.7us of issue time on its engine, so DMA count
    on the critical engines is minimized),
  - bulk weight loads issue from the sync engine as two parallel chains
    (a single DMA stream only sustains ~100-150GB/s; two keep the HBM
    path saturated); small consts issue from the scalar engine so the
    activation-table loads are not queued behind bulk issue,
  - attention pooling runs in column space: transpose the 96 gathered
    attention rows once on the PE, then mention-mean / head-product /
    head-mean are full-width DVE ops,
  - rs = seq^T @ ht uses ht as the stationary operand (8 matmuls of
    N<=512 per batch instead of 48 matmuls of N=1),
  - ht normalization is folded out of the critical path,
  - extractor accumulation is split into arrival-ordered phases that
    pipeline with the weight DMA chains; classifier is split so its
    first half starts as soon as wbsA lands.
"""

import sys

for _p in ("/opt/trn_rl_repo",):
    if _p not in sys.path:
        sys.path.insert(0, _p)

import numpy as np
import ml_dtypes

import concourse.bacc as bacc
import concourse.bass as bass
import concourse.mybir as mybir
from concourse.tile import TileContext
from concourse.bass_utils import run_bass_kernel_spmd
from concourse.tile_rust import add_dep_helper

F32 = mybir.dt.float32
BF16 = mybir.dt.bfloat16
I32 = mybir.dt.int32
AF = mybir.ActivationFunctionType
ALU = mybir.AluOpType
AX = mybir.AxisListType

B, L, HID = 16, 512, 768
HEADS, M = 12, 4
EMB, BLK, NER, NCLS = 768, 8, 6, 97
NCORES = 8
BPC = B // NCORES          # batches per core = 2
CAT = 2 * HID + NER        # 1542
NEMB = EMB // 128          # 6 chunks of EMB
NL = L // 128              # 4 chunks of L
NBL = EMB * BLK // 128     # 48 classifier contraction chunks
HALVES = ((0, 384), (384, 384))

# cbs bf16 [128, 103]: id96 | selE | ones | sel2
CBS_ID96 = 0
CBS_SELE = 96
CBS_ONES = 100
CBS_SEL2 = 101
CBS_COLS = 103
# cbias bf16 [1, 1544]: bh | selbh | bt | selbt (single partition row)
CB_BH = 0
CB_SELBH = EMB
CB_BT = EMB + 4
CB_SELBT = 2 * EMB + 4
CB_COLS = 2 * EMB + 8
# smallf f32 [97, 10]: id4 | bb col | ner [6,4]
SMF_BB = 4
SMF_NER = 5
SMF_COLS = 10

_cache = {}


def _build_constants():
    cbs = np.zeros((128, CBS_COLS), ml_dtypes.bfloat16)
    cbs[0:96, 0:96] = np.eye(96)
    for k in range(4 * M):
        cbs[k, CBS_SELE + k // M] = 1.0
    cbs[0:128, CBS_ONES] = 1.0
    cbs[0:4, CBS_SEL2] = 1.0
    cbs[4:8, CBS_SEL2 + 1] = 1.0

    rys = np.zeros((128, BLK * 128), ml_dtypes.bfloat16)
    for y in range(BLK):
        for p in range(128):
            rys[(p // BLK) * BLK + y, y * 128 + p] = 1.0

    perm = np.empty(EMB * BLK, np.int64)
    for cch in range(NEMB):
        for y in range(BLK):
            for p in range(128):
                g = cch * 16 + p // BLK
                x = p % BLK
                perm[(cch * BLK + y) * 128 + p] = g * 64 + x * BLK + y
    return {"cbs": cbs, "rys": rys, "perm": perm}


def _build_program():
    nc = bacc.Bacc("TRN2", target_bir_lowering=False, debug=False)

    seq_h = nc.dram_tensor("seq", [BPC * L, HID], BF16, kind="ExternalInput")
    attn_h = nc.dram_tensor("attn", [BPC * HEADS * L, L], BF16, kind="ExternalInput")
    idxp_h = nc.dram_tensor("idxp", [128, 2], I32, kind="ExternalInput")
    smf_h = nc.dram_tensor("smf", [NCLS, SMF_COLS], F32, kind="ExternalInput")
    cbs_h = nc.dram_tensor("cbs", [128, CBS_COLS], BF16, kind="ExternalInput")
    cbias_h = nc.dram_tensor("cbias", [1, CB_COLS], BF16, kind="ExternalInput")
    rys_h = nc.dram_tensor("rys", [128, BLK * 128], BF16, kind="ExternalInput")
    whsA_h = nc.dram_tensor("whsA", [128, 6 * EMB], BF16, kind="ExternalInput")
    whsB_h = nc.dram_tensor("whsB", [128, 6 * EMB], BF16, kind="ExternalInput")
    whn_h = nc.dram_tensor("whn", [NER, EMB], BF16, kind="ExternalInput")
    wtsA_h = nc.dram_tensor("wtsA", [128, 6 * EMB], BF16, kind="ExternalInput")
    wtsB_h = nc.dram_tensor("wtsB", [128, 6 * EMB], BF16, kind="ExternalInput")
    wtn_h = nc.dram_tensor("wtn", [NER, EMB], BF16, kind="ExternalInput")
    wbsA_h = nc.dram_tensor("wbsA", [128, 24 * NCLS], BF16, kind="ExternalInput")
    wbsB_h = nc.dram_tensor("wbsB", [128, 24 * NCLS], BF16, kind="ExternalInput")
    out_h = nc.dram_tensor("logitsT", [NCLS, BPC], F32, kind="ExternalOutput")

    with TileContext(nc) as tc:
        with (
            tc.tile_pool(name="const", bufs=1) as cp,
            tc.tile_pool(name="data", bufs=1) as dp,
            tc.tile_pool(name="psbig", bufs=1, space="PSUM") as psb,
            tc.tile_pool(name="psrs", bufs=1, space="PSUM") as psr,
            tc.tile_pool(name="pssm", bufs=1, space="PSUM") as pss,
        ):
            # ---- sync engine: seq first, then the two bulk chains ----
            seqt = []
            for b in range(BPC):
                t = dp.tile([128, NL * HID], BF16, tag=f"seq{b}")
                nc.sync.dma_start(
                    t[:].rearrange("p (c d) -> p c d", c=NL),
                    seq_h[b * L:(b + 1) * L, :].rearrange("(c p) d -> p c d", p=128))
                seqt.append(t)
            whsA = cp.tile([128, 6 * EMB], BF16)
            d_whsA = nc.sync.dma_start(whsA[:], whsA_h[:])
            wtsA = cp.tile([128, 6 * EMB], BF16)
            d_wtsA = nc.sync.dma_start(wtsA[:], wtsA_h[:])
            whsB = cp.tile([128, 6 * EMB], BF16)
            d_whsB = nc.sync.dma_start(whsB[:], whsB_h[:])
            wtsB = cp.tile([128, 6 * EMB], BF16)
            d_wtsB = nc.sync.dma_start(wtsB[:], wtsB_h[:])
            rys = cp.tile([128, BLK * 128], BF16)
            d_rys = nc.sync.dma_start(rys[:], rys_h[:])
            wbsA = cp.tile([128, 24 * NCLS], BF16)
            d_wbsA = nc.sync.dma_start(wbsA[:], wbsA_h[:])
            wbsB = cp.tile([128, 24 * NCLS], BF16)
            d_wbsB = nc.sync.dma_start(wbsB[:], wbsB_h[:])
            # two chains keep >=2 bulk streams in flight the whole window
            for a, b2 in ((d_whsB, d_whsA), (d_rys, d_whsB), (d_wbsA, d_rys),
                          (d_wtsB, d_wtsA), (d_wbsB, d_wtsB)):
                add_dep_helper(a.ins, b2.ins, reason="stagger bulk DMA bandwidth")

            # ---- scalar engine: packed small loads, then activations ----
            idxp = dp.tile([128, 2], I32)
            nc.scalar.dma_start(idxp[:], idxp_h[:])
            cbs = cp.tile([128, CBS_COLS], BF16)
            nc.scalar.dma_start(cbs[:], cbs_h[:])
            cbias = cp.tile([1, CB_COLS], BF16)
            nc.scalar.dma_start(cbias[:], cbias_h[:])
            smf = cp.tile([NCLS, SMF_COLS], F32)
            nc.scalar.dma_start(smf[:], smf_h[:])
            whn = cp.tile([NER, EMB], BF16)
            nc.scalar.dma_start(whn[:], whn_h[:])
            wtn = cp.tile([NER, EMB], BF16)
            nc.scalar.dma_start(wtn[:], wtn_h[:])

            id96 = cbs[0:96, CBS_ID96:CBS_ID96 + 96]
            selE = cbs[0:16, CBS_SELE:CBS_SELE + 4]
            ones = cbs[0:128, CBS_ONES:CBS_ONES + 1]
            sel2 = cbs[0:8, CBS_SEL2:CBS_SEL2 + 2]
            id4f = smf[0:4, 0:4]
            bbc = smf[0:NCLS, SMF_BB:SMF_BB + 1]
            ner4f = smf[0:NER, SMF_NER:SMF_NER + 4]
            bhr = cbias[0:1, CB_BH:CB_BH + EMB]
            btr = cbias[0:1, CB_BT:CB_BT + EMB]
            selbh = cbias[0:1, CB_SELBH:CB_SELBH + 4]
            selbt = cbias[0:1, CB_SELBT:CB_SELBT + 4]

            # ---- gathers (gpsimd; issue as soon as idxp lands) ----
            gather_ins = []
            at = []
            for b in range(BPC):
                t = dp.tile([2 * M * HEADS, L], BF16, tag=f"at{b}")
                off = idxp[16:112, 0:1] if b == 0 else idxp[0:96, 1:2]
                gather_ins.append(nc.gpsimd.indirect_dma_start(
                    out=t[:], out_offset=None, in_=attn_h[:],
                    in_offset=bass.IndirectOffsetOnAxis(ap=off, axis=0)))
                at.append(t)
            sg = dp.tile([4 * M, HID], BF16)
            gather_ins.append(nc.gpsimd.indirect_dma_start(
                out=sg[:], out_offset=None, in_=seq_h[:],
                in_offset=bass.IndirectOffsetOnAxis(ap=idxp[0:16, 0:1], axis=0)))

            # bulk weight chains yield the DMA path to the attention gathers
            for g in gather_ins[:2]:
                add_dep_helper(d_whsA.ins, g.ins,
                               reason="bulk yields DMA bandwidth to gathers")
                add_dep_helper(d_wtsA.ins, g.ins,
                               reason="bulk yields DMA bandwidth to gathers")

            ner4 = dp.tile([NER, 4], BF16)
            nc.vector.tensor_copy(ner4[:], ner4f)

            # PE p-state warm-up bridge (full clock needs ~3us continuous)
            ps_warm = pss.tile([96, 96], F32, tag="s1", name="ps_warm")
            for w in range(12):
                nc.tensor.matmul(ps_warm[:], lhsT=id96, rhs=id96,
                                 start=(w == 0), stop=(w == 11))

            # ---- attention pooling in column space ----
            # atT free layout: (b, c, q) with q = e*48 + m*12 + h
            atT = pss.tile([128, BPC * NL * 96], BF16, tag="s0")
            for b in range(BPC):
                for c in range(NL):
                    nc.tensor.transpose(
                        atT[:, (b * NL + c) * 96:(b * NL + c) * 96 + 96],
                        at[b][:, c * 128:(c + 1) * 128], id96)

            # ---- entity embeddings: log-sum-exp over mentions ----
            exps = dp.tile([4 * M, HID], BF16)
            nc.scalar.activation(exps[:], sg[:], AF.Exp)
            ps_e = [psb.tile([4, 384], F32, tag=f"wh{i}", name=f"ps_e{i}")
                    for i in range(2)]
            ent = dp.tile([4, HID], BF16)
            for i, (n0, nl_) in enumerate(HALVES):
                nc.tensor.matmul(ps_e[i][:], lhsT=selE,
                                 rhs=exps[:, n0:n0 + nl_], start=True, stop=True)
                nc.scalar.activation(ent[:, n0:n0 + nl_], ps_e[i][:], AF.Ln)

            # mean over mentions (scales folded into the final normalization)
            ea = dp.tile([128, 8 * 2 * HEADS], F32)
            nc.vector.tensor_reduce(
                ea[:].rearrange("p (bc e h) -> p bc e h", bc=8, e=2),
                atT[:].rearrange("p (bc e m h) -> p bc e h m", bc=8, e=2, m=M),
                axis=AX.X, op=ALU.add)
            hh = dp.tile([128, 8 * HEADS], BF16)
            nc.vector.tensor_tensor(
                out=hh[:].rearrange("p (bc h) -> p bc h", bc=8),
                in0=ea[:].rearrange("p (bc e h) -> p bc e h", bc=8, e=2)[:, :, 0],
                in1=ea[:].rearrange("p (bc e h) -> p bc e h", bc=8, e=2)[:, :, 1],
                op=ALU.mult)
            htr = dp.tile([128, 8], F32)
            nc.vector.tensor_reduce(
                htr[:].rearrange("p (bc one) -> p bc one", one=1),
                hh[:].rearrange("p (bc h) -> p bc h", bc=8),
                axis=AX.X, op=ALU.add)
            htc = dp.tile([128, 8], BF16)
            nc.vector.tensor_copy(htc[:], htr[:])

            # ---- denominator path (off the rs critical path) ----
            ps_d8 = pss.tile([8, 1], F32, tag="s1")
            nc.tensor.matmul(ps_d8[:], lhsT=htc[:], rhs=ones, start=True, stop=True)
            d8 = dp.tile([8, 1], BF16)
            nc.vector.tensor_copy(d8[:], ps_d8[:])
            ps_d2 = pss.tile([1, 2], F32, tag="s1")
            nc.tensor.matmul(ps_d2[:], lhsT=d8[:], rhs=sel2, start=True, stop=True)
            den2 = dp.tile([1, 2], F32)
            # den = sum(ht_raw) + M^2*HEADS*1e-5  (all scales folded here)
            nc.vector.tensor_scalar_add(den2[:], ps_d2[:], M * M * HEADS * 1e-5)
            rcp = dp.tile([1, 2], F32)
            nc.vector.reciprocal(rcp[:], den2[:])

            # ---- rs (unnormalized) = seq^T @ ht_raw, then rescale ----
            rsraw = dp.tile([1, BPC * HID], F32)
            ps_rst = pss.tile([128, 2 * NEMB], F32, tag="s0")
            for b in range(BPC):
                ps_rs = psr.tile([1, HID], F32, tag="rs", name=f"ps_rs{b}")
                for n0, nl_ in ((0, 512), (512, 256)):
                    for c in range(NL):
                        nc.tensor.matmul(
                            ps_rs[:, n0:n0 + nl_],
                            lhsT=htc[:, b * NL + c:b * NL + c + 1],
                            rhs=seqt[b][:, c * HID + n0:c * HID + n0 + nl_],
                            start=(c == 0), stop=(c == NL - 1))
                if b == 0:
                    nc.scalar.activation(
                        rsraw[0:1, 0:HID], ps_rs[:], AF.Copy,
                        scale=rcp[0:1, 0:1])
                else:
                    nc.vector.tensor_scalar_mul(
                        rsraw[0:1, b * HID:(b + 1) * HID], ps_rs[:],
                        rcp[0:1, b:b + 1])
                for c in range(NEMB):
                    nc.tensor.transpose(
                        ps_rst[:, c * 2 + b:c * 2 + b + 1],
                        rsraw[0:1, b * HID + c * 128:b * HID + (c + 1) * 128],
                        id4f[0:1, 0:1])

            # ---- ent transposes to columns (after Ln, before extractor) ----
            ps_et = pss.tile([128, 4 * NEMB], BF16, tag="s1")
            for c in range(NEMB):
                nc.tensor.transpose(ps_et[:, c * 4:(c + 1) * 4],
                                    ent[:, c * 128:(c + 1) * 128], id96[0:4, 0:4])
            entT = dp.tile([128, 4 * NEMB], BF16)
            nc.vector.tensor_copy(entT[:], ps_et[:])

            rsc = dp.tile([128, 4 * NEMB], BF16)
            nc.vector.tensor_copy(
                rsc[:].rearrange("p (r b m) -> p r b m", r=NEMB, b=BPC),
                ps_rst[:].rearrange("p (r b) -> p r b", r=NEMB)
                .unsqueeze(3).broadcast_to([128, NEMB, BPC, 2]))

            # ---- extractor GEMMs, pipelined with the weight DMA chains ----
            ps_wh = [psb.tile([4, 384], F32, tag=f"wh{i}", name=f"ps_wh{i}")
                     for i in range(2)]
            ps_wt = [psb.tile([4, 384], F32, tag=f"wt{i}", name=f"ps_wt{i}")
                     for i in range(2)]
            exts = ((ps_wh, whsA, whsB, whn, selbh, bhr),
                    (ps_wt, wtsA, wtsB, wtn, selbt, btr))
            # phase A: ner + bias + hs chunks (needs only *A weights + entT)
            for ps_w, wA, wB, wn, selb, br in exts:
                for i, (n0, nl_) in enumerate(HALVES):
                    nc.tensor.matmul(ps_w[i][:], lhsT=ner4[:],
                                     rhs=wn[:, n0:n0 + nl_], start=True, stop=False)
                    nc.tensor.matmul(ps_w[i][:], lhsT=selb,
                                     rhs=br[:, n0:n0 + nl_], start=False, stop=False)
                    for j in range(NEMB):
                        nc.tensor.matmul(
                            ps_w[i][:], lhsT=entT[:, j * 4:(j + 1) * 4],
                            rhs=wA[:, j * EMB + n0:j * EMB + n0 + nl_],
                            start=False, stop=False)
            # phase B: rs chunks (needs *B weights + rsc)
            t4 = []
            for wi, (ps_w, wA, wB, wn, selb, br) in enumerate(exts):
                t = dp.tile([4, EMB], BF16, tag=f"t4_{wi}")
                for i, (n0, nl_) in enumerate(HALVES):
                    for j in range(NEMB):
                        nc.tensor.matmul(
                            ps_w[i][:], lhsT=rsc[:, j * 4:(j + 1) * 4],
                            rhs=wB[:, j * EMB + n0:j * EMB + n0 + nl_],
                            start=False, stop=(j == NEMB - 1))
                    nc.scalar.activation(t[:, n0:n0 + nl_], ps_w[i][:], AF.Tanh)
                t4.append(t)

            # ---- transpose hs2/ts2 to columns ----
            ps_a = pss.tile([128, 4 * NEMB], BF16, tag="s0")
            ps_b2 = pss.tile([128, 4 * NEMB], BF16, tag="s1")
            for c in range(NEMB):
                nc.tensor.transpose(ps_a[:, c * 4:(c + 1) * 4],
                                    t4[0][:, c * 128:(c + 1) * 128], id96[0:4, 0:4])
                nc.tensor.transpose(ps_b2[:, c * 4:(c + 1) * 4],
                                    t4[1][:, c * 128:(c + 1) * 128], id96[0:4, 0:4])
            h2t = dp.tile([128, 4 * NEMB], BF16)
            nc.vector.tensor_copy(
                h2t[:].rearrange("p (c b) -> p c b", c=NEMB)[:, :, 0:4:2],
                ps_a[:].rearrange("p (c b) -> p c b", c=NEMB)[:, :, 0:4:2])
            nc.vector.tensor_copy(
                h2t[:].rearrange("p (c b) -> p c b", c=NEMB)[:, :, 1:4:2],
                ps_b2[:].rearrange("p (c b) -> p c b", c=NEMB)[:, :, 1:4:2])

            # ---- grouped bilinear ----
            # ts-replication: out col layout (y, c, b) = y*12 + c*2 + b
            ps_t2x = pss.tile([128, BLK * NEMB * BPC], F32, tag="s1")
            tscols = h2t[:].rearrange("p (c b) -> p c b", c=NEMB)[:, :, 1:4:2]
            for y in range(BLK):
                nc.tensor.matmul(
                    ps_t2x[:, y * 12:(y + 1) * 12]
                    .rearrange("p (c b) -> p c b", c=NEMB),
                    lhsT=rys[:, y * 128:(y + 1) * 128],
                    rhs=tscols, start=True, stop=True)
            # blt col layout (c, y, b)
            blt = dp.tile([128, NEMB * 16], BF16)
            nc.vector.tensor_tensor(
                out=blt[:].rearrange("p (c y b) -> p c y b", c=NEMB, y=BLK),
                in0=h2t[:].rearrange("p (c b) -> p c b", c=NEMB)[:, :, 0:4:2]
                .unsqueeze(3).broadcast_to([128, NEMB, BPC, BLK])
                .rearrange("p c b y -> p c y b"),
                in1=ps_t2x[:].rearrange("p (y c b) -> p c y b", y=BLK, c=NEMB),
                op=ALU.mult)

            # ---- classifier (split so half A starts when wbsA lands) ----
            ps_l = pss.tile([NCLS, BPC], F32, tag="s0")
            for hi, (half, wbs) in enumerate(((1, wbsB), (0, wbsA))):
                for kk in range(24):
                    k = half * 24 + kk
                    nc.tensor.matmul(ps_l[:], lhsT=wbs[:, kk * NCLS:(kk + 1) * NCLS],
                                     rhs=blt[:, k * 2:k * 2 + 2],
                                     start=(hi == 0 and kk == 0),
                                     stop=(hi == 1 and kk == 23))
            lg = dp.tile([NCLS, BPC], F32)
            nc.vector.tensor_scalar_add(lg[:], ps_l[:], bbc[:, :1])
            nc.scalar.dma_start(out_h[:], lg[:])

    nc.finalize()
    return nc


def _get_program():
    if "nc" not in _cache:
        _cache["nc"] = _build_program()
        _cache["consts"] = _build_constants()
    return _cache["nc"], _cache["consts"]


def kernel(sequence_output, attention, entity_pos, hs_ner_tags, ts_ner_tags,
           Wh, bh, Wt, bt, Wb, bb):
    nc, c = _get_program()

    seq = np.asarray(sequence_output, dtype=np.float32).astype(ml_dtypes.bfloat16)
    attn = np.asarray(attention, dtype=np.float32).astype(ml_dtypes.bfloat16)
    pos = np.asarray(entity_pos).astype(np.int64)
    nh = np.asarray(hs_ner_tags, dtype=np.float32)
    nt = np.asarray(ts_ner_tags, dtype=np.float32)
    whT = np.ascontiguousarray(np.asarray(Wh, dtype=np.float32).T).astype(ml_dtypes.bfloat16)
    wtT = np.ascontiguousarray(np.asarray(Wt, dtype=np.float32).T).astype(ml_dtypes.bfloat16)
    wbT = np.ascontiguousarray(np.asarray(Wb, dtype=np.float32).T)[c["perm"]]
    wbT = wbT.astype(ml_dtypes.bfloat16)

    def wimg(w, lo):
        # [128, (j, EMB)] image of contraction chunks lo..lo+5
        blk = w[lo * 128:(lo + 6) * 128]
        return np.ascontiguousarray(
            blk.reshape(6, 128, EMB).transpose(1, 0, 2).reshape(128, 6 * EMB))

    whsA, whsB = wimg(whT, 0), wimg(whT, 6)
    wtsA, wtsB = wimg(wtT, 0), wimg(wtT, 6)
    whn = np.ascontiguousarray(whT[12 * 128:CAT])
    wtn = np.ascontiguousarray(wtT[12 * 128:CAT])
    wbp = wbT.reshape(NBL, 128, NCLS).transpose(1, 0, 2)
    wbsA = np.ascontiguousarray(wbp[:, 0:24].reshape(128, 24 * NCLS))
    wbsB = np.ascontiguousarray(wbp[:, 24:48].reshape(128, 24 * NCLS))

    cbias = np.zeros((1, CB_COLS), ml_dtypes.bfloat16)
    cbias[0, CB_BH:CB_BH + EMB] = np.asarray(bh, np.float32)
    cbias[0, CB_BT:CB_BT + EMB] = np.asarray(bt, np.float32)
    cbias[0, CB_SELBH:CB_SELBH + 4] = np.array([1.0, 0.0, 1.0, 0.0])
    cbias[0, CB_SELBT:CB_SELBT + 4] = np.array([0.0, 1.0, 0.0, 1.0])

    in_maps = []
    for core in range(NCORES):
        b0 = core * BPC
        pc = pos[b0:b0 + BPC]                                # [2,2,M]
        ner = np.stack([nh[b0], nt[b0], nh[b0 + 1], nt[b0 + 1]], axis=1)
        smf = np.zeros((NCLS, SMF_COLS), np.float32)
        smf[0:4, 0:4] = np.eye(4)
        smf[0:NCLS, SMF_BB] = np.asarray(bb, np.float32)
        smf[0:NER, SMF_NER:SMF_NER + 4] = ner
        # idxp col0: seq gather rows (16) then attn rows for b=0 (96);
        # col1: attn rows for b=1
        idxp = np.zeros((128, 2), np.int32)
        for k in range(4 * M):
            b, e, m = k // 8, (k // 4) % 2, k % 4
            idxp[k, 0] = b * L + int(pc[b, e, m]) + 1
        for b in range(BPC):
            for q in range(2 * M * HEADS):
                e, m, h = q // 48, (q // 12) % 4, q % 12
                v = (b * HEADS + h) * L + int(pc[b, e, m]) + 1
                if b == 0:
                    idxp[16 + q, 0] = v
                else:
                    idxp[q, 1] = v
        im = {
            "seq": np.ascontiguousarray(seq[b0:b0 + BPC]).reshape(BPC * L, HID),
            "attn": np.ascontiguousarray(attn[b0:b0 + BPC]).reshape(BPC * HEADS * L, L),
            "idxp": idxp,
            "smf": smf,
            "whsA": whsA, "whsB": whsB, "whn": whn,
            "wtsA": wtsA, "wtsB": wtsB, "wtn": wtn,
            "wbsA": wbsA, "wbsB": wbsB,
            "cbs": c["cbs"], "cbias": cbias, "rys": c["rys"],
        }
        in_maps.append(im)

    res = run_bass_kernel_spmd(nc, in_maps, core_ids=list(range(NCORES)))
    _cache["last_res"] = res
    out = np.empty((B, NCLS), np.float32)
    for core in range(NCORES):
        out[core * BPC:(core + 1) * BPC] = res.results[core]["logitsT"].T
    return out
